# revision 1
# baseline (speedup 1.0000x reference)
"""Causal self-attention (N=2, S=4096, E=768, H=12) on 8 NeuronCores.

Sharding: batch x head-group. Core c handles batch n = c // 4 and heads
h0 = (c % 4) * 3 .. h0+2 (3 heads per core, 24 (n,h) pairs over 8 cores).

Per-core kernel (SPMD, identical program, per-core input values):
  inputs:  xT   [768, 4096]   x[n] transposed (host layout prep)
           wqk  [3, 768, 128] per head [Wq_h | Wk_h] column blocks
           wv   [768, 192]    Wv columns for the 3 heads
           bqk  [128, 3]      per head [bq_h; bk_h]
           bv   [1, 192]
  output:  outT [3, 64, 4096] per-head attention output, transposed

Algorithm per head (flash-style, scores never leave the chip):
  qT/kT  [64, 4096] via matmul(lhsT=W-block, rhs=xT-slab), duplicated into
         both partition halves so score matmuls can row-pack 2 chunks
         concurrently (K=64 contraction uses half the PE array).
  scores are computed TRANSPOSED: sT[sk-chunk 128, sq-slab 512] =
         matmul(lhsT=kT-chunk, rhs=qT-slab), 3 chunks batched per PSUM
         tile; exp on ScalarE in one [128, 1536] ACTIVATE (scale=1/8
         folded into its free affine); causal mask zeroed in-place by
         GPSIMD affine_select on diagonal chunks; the PV matmul consumes
         exp(sT) directly as the streaming operand with lhsT = [v | 1]
         augmented value chunks, so the softmax row-sum accumulates for
         free in PSUM row 64.
  division by row-sum: DVE reciprocal -> 2KB DMA to partition 0 ->
         GPSIMD partition_broadcast -> DVE multiply, deferred to slab end
         and batched per stage so the in-order DVE queue never blocks.
All matmuls run as float32r (full PE rate at free-dim >= 256); emission is
software-pipelined across heads so ScalarE (the bottleneck engine at
~230us/core of exp work) stays saturated.
"""

import os
import sys

import numpy as np

for _p in ("/opt/trn_rl_repo",):
    if _p not in sys.path and os.path.isdir(_p):
        sys.path.insert(0, _p)

import concourse.bass as bass  # noqa: E402
import concourse.mybir as mybir  # noqa: E402
import concourse.tile as tile  # noqa: E402
from concourse import bacc  # noqa: E402

F32 = mybir.dt.float32
F32R = mybir.dt.float32r

N, S, E, H = 2, 4096, 768, 12
D = 64
HPC = 3  # heads per core
P = 128
SLAB = 512
CHUNK = 128
GROUP = 3  # score chunks per exp batch
KCH = E // P  # 6 contraction chunks


def build_nc(seq=S, n_cores=8, reps=1):
    """Build and compile the per-core Bass program (parameterized for tests).

    reps > 1 emits the whole computation multiple times (idempotent) so
    hardware time can be measured by wall-clock differencing when no
    profiler is available.
    """
    nslab = seq // SLAB
    nchunk = seq // CHUNK
    cps = SLAB // CHUNK  # chunks per slab

    nc = bacc.Bacc("TRN2", target_bir_lowering=False, debug=False,
                   num_devices=n_cores)

    xT_d = nc.dram_tensor("xT", [E, seq], F32R, kind="ExternalInput")
    wqk_d = nc.dram_tensor("wqk", [HPC, E, P], F32R, kind="ExternalInput")
    wv_d = nc.dram_tensor("wv", [E, 256], F32R, kind="ExternalInput")
    bqk_d = nc.dram_tensor("bqk", [P, HPC], F32, kind="ExternalInput")
    bv_d = nc.dram_tensor("bv", [1, HPC * D], F32, kind="ExternalInput")
    outT_d = nc.dram_tensor("outT", [HPC, D, seq], F32, kind="ExternalOutput")

    xT_r = xT_d.ap().rearrange("(o p) s -> p o s", p=P)
    wqk_r = wqk_d.ap().rearrange("h (o p) m -> p h o m", p=P)
    wv_r = wv_d.ap().rearrange("(o p) m -> p o m", p=P)

    add = mybir.AluOpType.add
    mult = mybir.AluOpType.mult
    Exp = mybir.ActivationFunctionType.Exp

    with tile.TileContext(nc) as tc:
        with (
            tc.tile_pool(name="const", bufs=1) as cpool,
            tc.tile_pool(name="persist", bufs=1) as ppool,
            tc.tile_pool(name="xt", bufs=2) as xtpool,
            tc.tile_pool(name="ework", bufs=3) as epool,
            tc.tile_pool(name="small", bufs=2) as spool,
            tc.tile_pool(name="psc", bufs=2, space="PSUM") as psc,
            tc.tile_pool(name="ppv", bufs=1, space="PSUM") as ppv,
            tc.tile_pool(name="pproj", bufs=1, space="PSUM") as pproj,
        ):
            # ---- constants ----
            # wqk + the first x slab lead the SP DMA queue: they gate the
            # first score matmul, everything else has slack.
            wqk_sb = cpool.tile([P, HPC, KCH, P], F32R)
            nc.sync.dma_start(wqk_sb[:], wqk_r)
            xt_first = xtpool.tile([P, KCH, SLAB], F32R, tag="xt", name="xt")
            nc.sync.dma_start(xt_first[:], xT_r[:, :, 0:SLAB])
            wv_sb = cpool.tile([P, KCH, 256], F32R)
            nc.sync.dma_start(wv_sb[:], wv_r)
            bqk_sb = cpool.tile([P, HPC], F32)
            nc.sync.dma_start(bqk_sb[:], bqk_d.ap())
            bv1_sb = cpool.tile([1, HPC * D], F32)
            nc.sync.dma_start(bv1_sb[:], bv_d.ap())
            bv_bc = cpool.tile([P, HPC * D], F32)
            nc.gpsimd.partition_broadcast(bv_bc[:], bv1_sb[:])

            zeros_sb = cpool.tile([P, 3 * CHUNK], F32)
            nc.vector.memset(zeros_sb[:], 0.0)

            # 1-element dummy exp: forces the ACT table load at t=0, in
            # parallel with the input DMAs, instead of before the first
            # real exp on the critical path.
            warm = cpool.tile([1, 1], F32)
            nc.vector.memset(warm[:], 0.0)
            nc.scalar.activation(warm[:], warm[:], Exp)

            # [v | 1] augmented values: col D carries the softmax row-sum.
            v_aug = cpool.tile([P, nchunk, HPC, D + 1], F32R)
            ones_sb = cpool.tile([P, 1], F32)
            nc.vector.memset(ones_sb[:], 1.0)
            nc.vector.tensor_copy(
                v_aug[:, :, :, D : D + 1],
                ones_sb[:, None, None, :].to_broadcast((P, nchunk, HPC, 1)),
            )

            qdup = []
            kdup = []
            for h in range(HPC):
                qdup.append(ppool.tile([P, seq], F32R, name=f"qdup{h}"))
                kdup.append(ppool.tile([P, seq], F32R, name=f"kdup{h}"))

            def load_xt(j):
                sl = slice(j * SLAB, (j + 1) * SLAB)
                xt = xtpool.tile([P, KCH, SLAB], F32R, tag="xt", name="xt")
                nc.sync.dma_start(xt[:], xT_r[:, :, sl])
                return xt

            def proj_slab(j, xt):
                sl = slice(j * SLAB, (j + 1) * SLAB)
                for h in range(HPC):
                    ps = pproj.tile([P, SLAB], F32, tag="proj")
                    for k in range(KCH):
                        nc.tensor.matmul(
                            ps[:],
                            lhsT=wqk_sb[:, h, k, :],
                            rhs=xt[:, k, :],
                            start=(k == 0),
                            stop=(k == KCH - 1),
                        )
                    # psum rows [q|k] -> SBUF with bias; DMA fills the
                    # opposite partition half of each duplicate.
                    nc.vector.tensor_scalar_add(
                        qdup[h][0:D, sl], ps[0:D, :], bqk_sb[0:D, h : h + 1]
                    )
                    nc.vector.tensor_scalar_add(
                        kdup[h][D:P, sl], ps[D:P, :], bqk_sb[D:P, h : h + 1]
                    )
                    nc.sync.dma_start(qdup[h][D:P, sl], qdup[h][0:D, sl])
                    nc.sync.dma_start(kdup[h][0:D, sl], kdup[h][D:P, sl])
                for c4 in range(cps):
                    c = j * cps + c4
                    pv_ = pproj.tile([P, SLAB], F32, tag="proj")
                    for k in range(KCH):
                        nc.tensor.matmul(
                            pv_[:, 0:256],
                            lhsT=xt[:, k, c4 * CHUNK : (c4 + 1) * CHUNK],
                            rhs=wv_sb[:, k, :],
                            start=(k == 0),
                            stop=(k == KCH - 1),
                        )
                    nc.vector.tensor_tensor(
                        v_aug[:, c, :, 0:D],
                        pv_[:, 0 : HPC * D].rearrange("p (h d) -> p h d", h=HPC),
                        bv_bc[:].rearrange("p (h d) -> p h d", h=HPC),
                        add,
                    )

            def attn_units(h, j):
                """Return (scores_fn, pv_fn, tail_fn|None) triples for one
                head's slab; emission is pipelined across heads by the
                caller so ScalarE never waits at head boundaries."""
                sl = slice(j * SLAB, (j + 1) * SLAB)
                nch = (j + 1) * cps  # causal: key chunks 0 .. (j+1)*cps-1
                ngrp = (nch + GROUP - 1) // GROUP
                state = {}

                def scores_group(g):
                    c0 = g * GROUP
                    cn = min(GROUP, nch - c0)
                    sc = psc.tile([P, GROUP * SLAB], F32, tag="sc", name="sc")
                    for ci in range(c0, c0 + cn):
                        hb = D * (ci % 2)  # row-pack parity half
                        off = (ci - c0) * SLAB
                        nc.tensor.matmul(
                            sc[:, off : off + SLAB],
                            lhsT=kdup[h][
                                hb : hb + D, ci * CHUNK : (ci + 1) * CHUNK
                            ],
                            rhs=qdup[h][hb : hb + D, sl],
                            start=True,
                            stop=True,
                        )
                    et = epool.tile([P, GROUP * SLAB], F32R, tag="E", name="et")
                    # exp skips the fully-invalid prefix [0, 128m) of
                    # diagonal chunks (m >= 1); the affine_select below
                    # covers that prefix with fill=0.0 (its predicate is
                    # false there for every partition), so E never holds
                    # uninitialized data.
                    n_batch = sum(1 for ci in range(c0, c0 + cn)
                                  if ci <= j * cps)
                    if n_batch:
                        nc.scalar.activation(
                            et[:, : n_batch * SLAB], sc[:, : n_batch * SLAB],
                            Exp, scale=0.125,
                        )
                    for ci in range(c0, c0 + cn):
                        m = ci - j * cps
                        off = (ci - c0) * SLAB
                        if m >= 1:
                            nc.scalar.activation(
                                et[:, off + CHUNK * m : off + SLAB],
                                sc[:, off + CHUNK * m : off + SLAB],
                                Exp, scale=0.125,
                            )
                        if m >= 1:  # fully-invalid prefix -> zeros
                            nc.vector.tensor_copy(
                                et[:, off : off + CHUNK * m],
                                zeros_sb[:, : CHUNK * m],
                            )
                        if m >= 0:  # triangle: zero sq < sk entries
                            nc.gpsimd.affine_select(
                                out=et[:, off + CHUNK * m : off + CHUNK * (m + 1)],
                                in_=et[:, off + CHUNK * m : off + CHUNK * (m + 1)],
                                compare_op=mybir.AluOpType.is_ge,
                                fill=0.0,
                                base=0,
                                pattern=[[1, CHUNK]],
                                channel_multiplier=-1,
                            )
                    state[g] = (et, c0, cn)

                def pv_group(g):
                    if g == 0:
                        state["pv"] = ppv.tile([D + 1, SLAB], F32, tag="pv",
                                               name="pv")
                    pv = state["pv"]
                    et, c0, cn = state[g]
                    for ci in range(c0, c0 + cn):
                        off = (ci - c0) * SLAB
                        nc.tensor.matmul(
                            pv[:],
                            lhsT=v_aug[:, ci, h, :],
                            rhs=et[:, off : off + SLAB],
                            start=(ci == 0),
                            stop=(ci == nch - 1),
                            skip_group_check=True,
                        )

                def cp_fn():
                    # One DVE copy frees the PV psum bank immediately; the
                    # divide chain then runs from SBUF, deferred to slab end
                    # so its DVE->Pool->DVE ping-pong never head-of-line
                    # blocks the DVE queue mid-pipeline.
                    pv = state["pv"]
                    cp = spool.tile([D + 1, SLAB], F32, tag="cp", name="cp",
                                    bufs=3)
                    nc.vector.tensor_copy(cp[:], pv[:])
                    state["cp"] = cp

                def recip_fn():
                    cp = state["cp"]
                    nc.vector.reciprocal(cp[D : D + 1, :], cp[D : D + 1, :])

                def rec0_fn():
                    # HW partition_broadcast sources partition 0; DVE's
                    # reciprocal is lane-locked to partition 64, so a 2KB
                    # SBUF DMA hops the row down.
                    rec0 = spool.tile([1, SLAB], F32, tag="rec0", name="rec0",
                                      bufs=3)
                    nc.sync.dma_start(rec0[:], state["cp"][D : D + 1, :])
                    state["rec0"] = rec0

                def rbc_fn():
                    rbc = spool.tile([D, SLAB], F32, tag="rbc", name="rbc",
                                     bufs=3)
                    nc.gpsimd.partition_broadcast(rbc[:], state["rec0"][:])
                    state["rbc"] = rbc

                def mult_fn():
                    osb = spool.tile([D, SLAB], F32, tag="osb", name="osb",
                                     bufs=3)
                    nc.vector.tensor_tensor(
                        osb[:], state["cp"][0:D, :], state["rbc"][:], mult
                    )
                    nc.sync.dma_start(outT_d.ap()[h, :, sl], osb[:])

                units = [
                    [
                        (lambda g=g: scores_group(g)),
                        (lambda g=g: pv_group(g)),
                        None,
                    ]
                    for g in range(ngrp)
                ]
                units[-1][2] = cp_fn
                return units, (recip_fn, rec0_fn, rbc_fn, mult_fn)

            # Emission: xt DMA prefetched one slab ahead; within a slab all
            # heads' (scores, pv) groups form one software pipeline --
            # scores(i+1) is emitted before pv(i) so the in-order PE queue
            # always has score work while ScalarE exps group i.
            for _rep in range(reps):
                xt_cur = xt_first if _rep == 0 else load_xt(0)
                for j in range(nslab):
                    proj_slab(j, xt_cur)
                    if j + 1 < nslab:
                        xt_cur = load_xt(j + 1)
                    units = []
                    divs = []
                    for h in range(HPC):
                        u, dv = attn_units(h, j)
                        units.extend(u)
                        divs.append(dv)
                    units[0][0]()  # scores(0)
                    for i, (_, pv_fn, cp_fn) in enumerate(units):
                        if i + 1 < len(units):
                            units[i + 1][0]()  # scores(i+1)
                        pv_fn()
                        if cp_fn is not None:
                            cp_fn()
                    # deferred division chains, batched per stage so the
                    # in-order DVE queue never waits on the Pool broadcast
                    for stage in range(4):
                        for dv in divs:
                            dv[stage]()

    nc.compile()
    return nc


def shard_inputs(x, Wq, bq, Wk, bk, Wv, bv, n_cores=8, hpc=HPC):
    """Host-side layout prep: slice per-core head groups + transpose x."""
    in_maps = []
    nb = x.shape[0]
    groups = n_cores // nb  # head groups per batch
    xT = [np.ascontiguousarray(x[n].T.astype(np.float32)) for n in range(nb)]
    for core in range(n_cores):
        n = core // groups
        h0 = (core % groups) * hpc
        wqk = np.stack(
            [
                np.concatenate(
                    [
                        Wq[:, (h0 + i) * D : (h0 + i + 1) * D],
                        Wk[:, (h0 + i) * D : (h0 + i + 1) * D],
                    ],
                    axis=1,
                )
                for i in range(hpc)
            ]
        ).astype(np.float32)
        bqk = np.stack(
            [
                np.concatenate(
                    [bq[(h0 + i) * D : (h0 + i + 1) * D],
                     bk[(h0 + i) * D : (h0 + i + 1) * D]]
                )
                for i in range(hpc)
            ],
            axis=1,
        ).astype(np.float32)
        in_maps.append(
            {
                "xT": xT[n],
                "wqk": np.ascontiguousarray(wqk),
                "wv": np.ascontiguousarray(
                    np.pad(
                        Wv[:, h0 * D : (h0 + hpc) * D].astype(np.float32),
                        ((0, 0), (0, 256 - hpc * D)),
                    )
                ),
                "bqk": np.ascontiguousarray(bqk),
                "bv": np.ascontiguousarray(
                    bv[None, h0 * D : (h0 + hpc) * D].astype(np.float32)
                ),
            }
        )
    return in_maps


def gather_output(results, n_cores=8, nb=N, seq=S, emb=E, hpc=HPC):
    out = np.empty((nb, seq, emb), np.float32)
    groups = n_cores // nb
    for core in range(n_cores):
        n = core // groups
        h0 = (core % groups) * hpc
        oT = results[core]["outT"]  # [hpc, D, seq]
        for i in range(hpc):
            out[n, :, (h0 + i) * D : (h0 + i + 1) * D] = oT[i].T
    return out


_NC_CACHE = {}


def _get_nc():
    if "nc" not in _NC_CACHE:
        _NC_CACHE["nc"] = build_nc()
    return _NC_CACHE["nc"]


def run_on_hw(inputs, trace=False):
    """Run on the 8 NeuronCores; returns (full_output, BassKernelResults)."""
    from concourse.bass_utils import run_bass_kernel_spmd

    nc = _get_nc()
    in_maps = shard_inputs(**inputs)
    res = run_bass_kernel_spmd(nc, in_maps, list(range(8)), trace=trace)
    return gather_output(res.results), res


def kernel(x, Wq, bq, Wk, bk, Wv, bv):
    x = np.asarray(x)
    out, _ = run_on_hw(
        dict(x=x, Wq=np.asarray(Wq), bq=np.asarray(bq), Wk=np.asarray(Wk),
             bk=np.asarray(bk), Wv=np.asarray(Wv), bv=np.asarray(bv))
    )
    return out.astype(np.float32)



# revision 6
# speedup vs baseline: 1.6884x; 1.6884x over previous
"""Causal self-attention (N=2, S=4096, E=768, H=12) on 8 NeuronCores.

Sharding: batch x head-group. Core c handles batch n = c // 4 and heads
h0 = (c % 4) * 3 .. h0+2 (3 heads per core, 24 (n,h) pairs over 8 cores).

v2 vs the original baseline: the exp wall (26M score elements/core,
previously all on ScalarE at ~153 G elem/s = the 165us bottleneck) is
split across engines: ScalarE keeps a tunable share as true exp
(activation Exp, bf16 out); DVE and Pool compute their shares with a
one-op Schraudolph: out_i16 = round(s'' + B) bitcast bf16 == exp(s - SHIFT)
to ~2% (the A16 = 2^7/ln2 factor is folded into k's projection bias-add,
so the exp op is a single tensor_scalar add). All operands are bf16
(xT, q/k dups, et, v_aug) which halves DMA and SBUF while keeping matmul
rate (1 row/cycle); accuracy budget ~1% total vs the 2e-2 gate.
Everything else follows the baseline: transposed flash-style scores with
2-chunk PE row-packing, [v | 1] augmented PV so the softmax row-sum rides
the PV matmul, GPSIMD affine_select causal masking, deferred division.
"""

import os
import sys

import numpy as np

for _p in ("/opt/trn_rl_repo",):
    if _p not in sys.path and os.path.isdir(_p):
        sys.path.insert(0, _p)

import concourse.bass as bass  # noqa: E402
import concourse.mybir as mybir  # noqa: E402
import concourse.tile as tile  # noqa: E402
from concourse import bacc  # noqa: E402

F32 = mybir.dt.float32
F32R = mybir.dt.float32r
BF16 = mybir.dt.bfloat16
I16 = mybir.dt.int16

N, S, E, H = 2, 4096, 768, 12
D = 64
HPC = 3  # heads per core
P = 128
SLAB = 512
CHUNK = 128
GROUP = 3  # score chunks per psum tile
KCH = E // P  # 6 contraction chunks

A16 = 128.0 / float(np.log(2.0))  # 2^7/ln2: bf16-Schraudolph slope
SHIFT = 2.0  # exp(s - SHIFT): cancels in the division, tames weight range
B16 = float(127 << 7) - A16 * SHIFT  # bf16 exponent bias - shift fold
# per-chunk exp engine pattern: 'S'=ScalarE true exp, 'D'=DVE schraudolph,
# 'P'=Pool schraudolph. ~60/25/15 keeps schraudolph share ~40%.
EXP_PATTERN = "SDSSDSSDSD"


def build_nc(seq=S, n_cores=8, reps=1):
    nslab = seq // SLAB
    cps = SLAB // CHUNK  # chunks per slab

    nc = bacc.Bacc("TRN2", target_bir_lowering=False, debug=False,
                   num_devices=n_cores)

    xT_d = nc.dram_tensor("xT", [E, seq], BF16, kind="ExternalInput")
    wqk_d = nc.dram_tensor("wqk", [HPC, E, P], BF16, kind="ExternalInput")
    wv_d = nc.dram_tensor("wv", [E, HPC * D], BF16, kind="ExternalInput")
    bqk_d = nc.dram_tensor("bqk", [P, HPC], F32, kind="ExternalInput")
    bv_d = nc.dram_tensor("bv", [1, HPC * D], F32, kind="ExternalInput")
    outT_d = nc.dram_tensor("outT", [HPC, D, seq], F32, kind="ExternalOutput")

    xT_r = xT_d.ap().rearrange("(o p) s -> p o s", p=P)
    wqk_r = wqk_d.ap().rearrange("h (o p) m -> p h o m", p=P)
    wv_r = wv_d.ap().rearrange("(o p) m -> p o m", p=P)

    add = mybir.AluOpType.add
    mult = mybir.AluOpType.mult
    Exp = mybir.ActivationFunctionType.Exp

    with tile.TileContext(nc) as tc:
        with (
            tc.tile_pool(name="const", bufs=1) as cpool,
            tc.tile_pool(name="persist", bufs=1) as ppool,
            tc.tile_pool(name="xt", bufs=2) as xtpool,
            tc.tile_pool(name="ework", bufs=3) as epool,
            tc.tile_pool(name="small", bufs=2) as spool,
            tc.tile_pool(name="psc", bufs=2, space="PSUM") as psc,
            tc.tile_pool(name="ppv", bufs=1, space="PSUM") as ppv,
            tc.tile_pool(name="pproj", bufs=1, space="PSUM") as pproj,
        ):
            # ---- constants ----
            wqk_sb = cpool.tile([P, HPC, KCH, P], BF16)
            nc.sync.dma_start(wqk_sb[:], wqk_r)
            xt_first = xtpool.tile([P, KCH, SLAB], BF16, tag="xt", name="xt")
            nc.sync.dma_start(xt_first[:], xT_r[:, :, 0:SLAB])
            wv_sb = cpool.tile([P, KCH, HPC * D], BF16)
            nc.sync.dma_start(wv_sb[:], wv_r)
            bqk_sb = cpool.tile([P, HPC], F32)
            nc.sync.dma_start(bqk_sb[:], bqk_d.ap())
            bv1_sb = cpool.tile([1, HPC * D], F32)
            nc.sync.dma_start(bv1_sb[:], bv_d.ap())
            bv_bc = cpool.tile([P, HPC * D], F32)
            nc.gpsimd.partition_broadcast(bv_bc[:], bv1_sb[:])

            zeros_sb = cpool.tile([P, 3 * CHUNK], BF16)
            nc.vector.memset(zeros_sb[:], 0.0)

            # 1-element dummy exp: load the ACT table at t=0
            warm = cpool.tile([1, 1], F32)
            nc.vector.memset(warm[:], 0.0)
            nc.scalar.activation(warm[:], warm[:], Exp)

            # per-partition -SHIFT bias for the ScalarE exp
            nshift_sb = cpool.tile([P, 1], F32)
            nc.vector.memset(nshift_sb[:], -SHIFT)

            # [v | 1] augmented values: col D carries the softmax row-sum.
            nchunk = seq // CHUNK
            v_aug = cpool.tile([P, nchunk, HPC, D + 1], BF16)
            ones_sb = cpool.tile([P, 1], BF16)
            nc.vector.memset(ones_sb[:], 1.0)
            nc.vector.tensor_copy(
                v_aug[:, :, :, D : D + 1],
                ones_sb[:, None, None, :].to_broadcast((P, nchunk, HPC, 1)),
            )

            qdup = []
            kdup = []
            for h in range(HPC):
                qdup.append(ppool.tile([P, seq], BF16, name=f"qdup{h}"))
                kdup.append(ppool.tile([P, seq], BF16, name=f"kdup{h}"))

            def load_xt(j):
                sl = slice(j * SLAB, (j + 1) * SLAB)
                xt = xtpool.tile([P, KCH, SLAB], BF16, tag="xt", name="xt")
                nc.sync.dma_start(xt[:], xT_r[:, :, sl])
                return xt

            def proj_slab(j, xt):
                sl = slice(j * SLAB, (j + 1) * SLAB)
                for h in range(HPC):
                    ps = pproj.tile([P, SLAB], F32, tag="proj")
                    for k in range(KCH):
                        nc.tensor.matmul(
                            ps[:],
                            lhsT=wqk_sb[:, h, k, :],
                            rhs=xt[:, k, :],
                            start=(k == 0),
                            stop=(k == KCH - 1),
                        )
                    # q = (q_psum + bq) in bf16; k = (k_psum + bk) * A16/8
                    # so the score matmul emits A16 * s directly.
                    nc.vector.tensor_scalar(
                        qdup[h][0:D, sl], ps[0:D, :],
                        bqk_sb[0:D, h : h + 1], None, add,
                    )
                    nc.scalar.activation(
                        kdup[h][D:P, sl], ps[D:P, :],
                        mybir.ActivationFunctionType.Identity,
                        bias=bqk_sb[D:P, h : h + 1], scale=A16 / 8.0,
                    )
                    nc.sync.dma_start(qdup[h][D:P, sl], qdup[h][0:D, sl])
                    nc.sync.dma_start(kdup[h][0:D, sl], kdup[h][D:P, sl])
                for c4 in range(cps):
                    c = j * cps + c4
                    pv_ = pproj.tile([P, SLAB], F32, tag="proj")
                    for k in range(KCH):
                        nc.tensor.matmul(
                            pv_[:, 0 : HPC * D],
                            lhsT=xt[:, k, c4 * CHUNK : (c4 + 1) * CHUNK],
                            rhs=wv_sb[:, k, :],
                            start=(k == 0),
                            stop=(k == KCH - 1),
                        )
                    nc.vector.tensor_tensor(
                        v_aug[:, c, :, 0:D],
                        pv_[:, 0 : HPC * D].rearrange("p (h d) -> p h d", h=HPC),
                        bv_bc[:].rearrange("p (h d) -> p h d", h=HPC),
                        add,
                    )

            def attn_units(h, j):
                sl = slice(j * SLAB, (j + 1) * SLAB)
                nch = (j + 1) * cps  # causal: key chunks 0 .. (j+1)*cps-1
                ngrp = (nch + GROUP - 1) // GROUP
                state = {}

                def exp_chunk(eng, et, sc, lo, hi):
                    """et[:, lo:hi] = exp(sc[:, lo:hi]/A16 - SHIFT).

                    sc holds A16*s (A16 folded into k). ScalarE does true
                    exp; DVE/Pool do int16-Schraudolph into the bf16 view.
                    """
                    if eng == "S":
                        nc.scalar.activation(
                            et[:, lo:hi], sc[:, lo:hi], Exp,
                            scale=1.0 / A16, bias=nshift_sb[:],
                        )
                    else:
                        nc.vector.tensor_scalar(
                            et[:, lo:hi].bitcast(I16), sc[:, lo:hi],
                            B16, None, add,
                        )

                def scores_group(g):
                    c0 = g * GROUP
                    cn = min(GROUP, nch - c0)
                    sc = psc.tile([P, GROUP * SLAB], F32, tag="sc", name="sc")
                    for ci in range(c0, c0 + cn):
                        hb = D * (ci % 2)  # row-pack parity half
                        off = (ci - c0) * SLAB
                        nc.tensor.matmul(
                            sc[:, off : off + SLAB],
                            lhsT=kdup[h][
                                hb : hb + D, ci * CHUNK : (ci + 1) * CHUNK
                            ],
                            rhs=qdup[h][hb : hb + D, sl],
                            start=True,
                            stop=True,
                        )
                    et = epool.tile([P, GROUP * SLAB], BF16, tag="E", name="et")
                    for ci in range(c0, c0 + cn):
                        m = ci - j * cps
                        off = (ci - c0) * SLAB
                        eng = EXP_PATTERN[(ci + j) % len(EXP_PATTERN)]
                        if m < 1:
                            exp_chunk(eng, et, sc, off, off + SLAB)
                        else:
                            exp_chunk(eng, et, sc, off + CHUNK * m, off + SLAB)
                            # fully-invalid prefix -> zeros
                            nc.gpsimd.tensor_copy(
                                et[:, off : off + CHUNK * m],
                                zeros_sb[:, : CHUNK * m],
                            )
                        if m >= 0:  # triangle: zero sq < sk entries
                            nc.gpsimd.affine_select(
                                out=et[:, off + CHUNK * m : off + CHUNK * (m + 1)],
                                in_=et[:, off + CHUNK * m : off + CHUNK * (m + 1)],
                                compare_op=mybir.AluOpType.is_ge,
                                fill=0.0,
                                base=0,
                                pattern=[[1, CHUNK]],
                                channel_multiplier=-1,
                            )
                    state[g] = (et, c0, cn)

                def pv_group(g):
                    if g == 0:
                        state["pv"] = ppv.tile([D + 1, SLAB], F32, tag="pv",
                                               name="pv")
                    pv = state["pv"]
                    et, c0, cn = state[g]
                    for ci in range(c0, c0 + cn):
                        off = (ci - c0) * SLAB
                        nc.tensor.matmul(
                            pv[:],
                            lhsT=v_aug[:, ci, h, :],
                            rhs=et[:, off : off + SLAB],
                            start=(ci == 0),
                            stop=(ci == nch - 1),
                            skip_group_check=True,
                        )

                def cp_fn():
                    pv = state["pv"]
                    cp = spool.tile([D + 1, SLAB], F32, tag="cp", name="cp",
                                    bufs=3)
                    nc.vector.tensor_copy(cp[:], pv[:])
                    state["cp"] = cp

                def recip_fn():
                    cp = state["cp"]
                    nc.vector.reciprocal(cp[D : D + 1, :], cp[D : D + 1, :])

                def rec0_fn():
                    rec0 = spool.tile([1, SLAB], F32, tag="rec0", name="rec0",
                                      bufs=3)
                    nc.sync.dma_start(rec0[:], state["cp"][D : D + 1, :])
                    state["rec0"] = rec0

                def rbc_fn():
                    rbc = spool.tile([D, SLAB], F32, tag="rbc", name="rbc",
                                     bufs=3)
                    nc.gpsimd.partition_broadcast(rbc[:], state["rec0"][:])
                    state["rbc"] = rbc

                def mult_fn():
                    osb = spool.tile([D, SLAB], F32, tag="osb", name="osb",
                                     bufs=3)
                    nc.gpsimd.tensor_tensor(
                        osb[:], state["cp"][0:D, :], state["rbc"][:], mult
                    )
                    nc.sync.dma_start(outT_d.ap()[h, :, sl], osb[:])

                units = [
                    [
                        (lambda g=g: scores_group(g)),
                        (lambda g=g: pv_group(g)),
                        None,
                    ]
                    for g in range(ngrp)
                ]
                units[-1][2] = cp_fn
                return units, (recip_fn, rec0_fn, rbc_fn, mult_fn)

            for _rep in range(reps):
                xt_cur = xt_first if _rep == 0 else load_xt(0)
                for j in range(nslab):
                    proj_slab(j, xt_cur)
                    if j + 1 < nslab:
                        xt_cur = load_xt(j + 1)
                    units = []
                    divs = []
                    for h in range(HPC):
                        u, dv = attn_units(h, j)
                        units.extend(u)
                        divs.append(dv)
                    units[0][0]()  # scores(0)
                    for i, (_, pv_fn, cp_fn) in enumerate(units):
                        if i + 1 < len(units):
                            units[i + 1][0]()  # scores(i+1)
                        pv_fn()
                        if cp_fn is not None:
                            cp_fn()
                    for stage in range(4):
                        for dv in divs:
                            dv[stage]()

    nc.compile()
    return nc


def shard_inputs(x, Wq, bq, Wk, bk, Wv, bv, n_cores=8, hpc=HPC):
    """Host-side layout prep: slice per-core head groups + transpose x."""
    import ml_dtypes

    bf16 = ml_dtypes.bfloat16
    in_maps = []
    nb = x.shape[0]
    groups = n_cores // nb  # head groups per batch
    xT = [np.ascontiguousarray(x[n].T.astype(bf16)) for n in range(nb)]
    for core in range(n_cores):
        n = core // groups
        h0 = (core % groups) * hpc
        wqk = np.stack(
            [
                np.concatenate(
                    [
                        Wq[:, (h0 + i) * D : (h0 + i + 1) * D],
                        Wk[:, (h0 + i) * D : (h0 + i + 1) * D],
                    ],
                    axis=1,
                )
                for i in range(hpc)
            ]
        ).astype(bf16)
        bqk = np.stack(
            [
                np.concatenate(
                    [bq[(h0 + i) * D : (h0 + i + 1) * D],
                     bk[(h0 + i) * D : (h0 + i + 1) * D] * (A16 / 8.0)]
                )
                for i in range(hpc)
            ],
            axis=1,
        ).astype(np.float32)
        in_maps.append(
            {
                "xT": xT[n],
                "wqk": np.ascontiguousarray(wqk),
                "wv": np.ascontiguousarray(
                    Wv[:, h0 * D : (h0 + hpc) * D].astype(bf16)
                ),
                "bqk": np.ascontiguousarray(bqk),
                "bv": np.ascontiguousarray(
                    bv[None, h0 * D : (h0 + hpc) * D].astype(np.float32)
                ),
            }
        )
    return in_maps


def gather_output(results, n_cores=8, nb=N, seq=S, emb=E, hpc=HPC):
    out = np.empty((nb, seq, emb), np.float32)
    groups = n_cores // nb
    for core in range(n_cores):
        n = core // groups
        h0 = (core % groups) * hpc
        oT = results[core]["outT"]  # [hpc, D, seq]
        for i in range(hpc):
            out[n, :, (h0 + i) * D : (h0 + i + 1) * D] = oT[i].T
    return out


_NC_CACHE = {}


def _get_nc():
    if "nc" not in _NC_CACHE:
        _NC_CACHE["nc"] = build_nc()
    return _NC_CACHE["nc"]


def run_on_hw(inputs, trace=False):
    """Run on the 8 NeuronCores; returns (full_output, BassKernelResults)."""
    from concourse.bass_utils import run_bass_kernel_spmd

    nc = _get_nc()
    in_maps = shard_inputs(**inputs)
    res = run_bass_kernel_spmd(nc, in_maps, list(range(8)), trace=trace)
    return gather_output(res.results), res


def kernel(x, Wq, bq, Wk, bk, Wv, bv):
    x = np.asarray(x)
    out, _ = run_on_hw(
        dict(x=x, Wq=np.asarray(Wq), bq=np.asarray(bq), Wk=np.asarray(Wk),
             bk=np.asarray(bk), Wv=np.asarray(Wv), bv=np.asarray(bv))
    )
    return out.astype(np.float32)
